# revision 29
# baseline (speedup 1.0000x reference)
"""Trainium2 Bass kernel for nn_Encoder_37915971289796 (6-layer transformer encoder).

Strategy: pure data-parallel over batch (B=16 -> 2 per core, 8 cores, no
collectives). Per core, activations live feature-major in SBUF; weights are
host-cast to bf16, host-relaid so every SBUF partition's data is one
contiguous 8KB HBM read, and streamed double-buffered; matmuls run bf16 with
fp32 PSUM accumulation.

v2 restructure vs the first working version:
- Attention: score matmuls write bf16 PSUM (half-width banks), the relative
  position bias is added by DVE at 2x bf16 rate straight into SBUF exp-input
  tiles; exp runs as two [128,2048] ACT calls per head pair; softmax
  denominators ride the context matmul stream as packed M=1 column-strip
  chains (no separate ones-matmul chains); a single reciprocal + one packed
  broadcast matmul pair + one multiply normalizes both heads. Score/exp work
  for iteration i+1 is emitted before the context matmuls of iteration i so
  the PE never waits on the scalar engine.
- LayerNorm: stats inputs are produced by DVE (bf16 copy + bf16 square at
  2x/4x modes) instead of big serial ACT passes; mean/E[x2] matmul chains are
  packed into one PSUM bank via column strips; 1/D is folded into the ones
  vector; normalization runs in bf16 at 2x DVE rate.
- Loop order: attention iterates b-outer so the output projection of the
  first 512 tokens overlaps attention of the second batch; QKV/FFN iterate
  ch-outer so each phase starts as soon as the first 512-token chunk of the
  previous phase is ready.
- QSCALE folded into wq host-side; ACT functions reduced to {Exp, Ln, Gelu,
  Copy} to minimize activation-table reloads.

Self-contained: hardcodes all shapes; takes FULL inputs, returns FULL output.
"""

import numpy as np
import ml_dtypes
from contextlib import ExitStack

import concourse.bass as bass
import concourse.mybir as mybir
import concourse.tile as tile
from concourse import bacc
from concourse.bass_utils import run_bass_kernel_spmd

F32 = mybir.dt.float32
BF16 = mybir.dt.bfloat16
FP8 = mybir.dt.float8e4
AF = mybir.ActivationFunctionType
DR = mybir.MatmulPerfMode.DoubleRow
BF = ml_dtypes.bfloat16
F8 = ml_dtypes.float8_e4m3fn if hasattr(ml_dtypes, "float8_e4m3fn") \
    else ml_dtypes.float8_e4m3
W8SCALE = 1024.0     # fp8 weight scale (power of 2, clip-safe)
X8SCALE = 32.0       # fp8 activation scale for xn / ctxT
EVSCALE = 1.0 / (W8SCALE * X8SCALE)   # folded into evacuations

L, D, H, F, S, B, P = 6, 1024, 16, 4096, 512, 16, 512
DH = D // H              # 64
NCORES = 8
BL = B // NCORES         # 2 batches per core
NT = BL * S              # 1024 tokens per core
DT = D // 128            # 8 d-tiles
FT = F // 128            # 32 f-tiles
TBL = 2 * P - 1          # 1023
GW = 896                 # flipped bias window width
EPS = 1e-6
QSCALE = 1.0 / float(np.sqrt(DH))

_CACHE = {}


def _build(flags):
    """Build the per-core Bass program. flags: (use_pbias, use_obias, use_ln1, use_ln2)"""
    use_pbias, use_obias, use_ln1, use_ln2 = flags
    nc = bacc.Bacc("TRN2", target_bir_lowering=False, debug=False)

    x_d = nc.dram_tensor("x", [BL, S, D], F32, kind="ExternalInput").ap()
    pe_d = nc.dram_tensor("pe", [S, D], F32, kind="ExternalInput").ap()
    # host-relayouted weights (bf16), per-partition contiguous:
    w_d = {}
    for w in ("wq", "wk", "wv", "wo"):
        w_d[w] = nc.dram_tensor(w, [L, 2, 128, DT, 512], FP8,
                                kind="ExternalInput").ap()
    w1_d = nc.dram_tensor("w1", [L, 8, 128, DT, 512], BF16,
                          kind="ExternalInput").ap()
    w2_d = nc.dram_tensor("w2", [L, DT, 128, FT, 128], BF16,
                          kind="ExternalInput").ap()
    # partition-flipped Toeplitz bias windows: gt[l,h,p,j] = table[l, 127-p+j, h]
    gt_d = nc.dram_tensor("gt", [L, H, 128, GW], BF16, kind="ExternalInput").ap()
    imat_d = nc.dram_tensor("imat", [128, 128], F32, kind="ExternalInput").ap()
    if use_pbias:  # bq*QSCALE, bk, bv, b1 applied via ACT bias APs
        pb_d = {w: nc.dram_tensor(f"b_{w}", [L, D if w != "b1" else F], F32,
                                  kind="ExternalInput").ap()
                for w in ("bq", "bk", "bv", "b1")}
    if use_obias:  # bo, b2 applied via extra DVE passes
        ob_d = {w: nc.dram_tensor(f"b_{w}", [L, D], F32, kind="ExternalInput").ap()
                for w in ("bo", "b2")}
    if use_ln1:
        ln1g_d = nc.dram_tensor("ln1_g", [L, D], F32, kind="ExternalInput").ap()
        ln1b_d = nc.dram_tensor("ln1_b", [L, D], F32, kind="ExternalInput").ap()
    if use_ln2:
        ln2g_d = nc.dram_tensor("ln2_g", [L, D], F32, kind="ExternalInput").ap()
        ln2b_d = nc.dram_tensor("ln2_b", [L, D], F32, kind="ExternalInput").ap()
    out_d = nc.dram_tensor("out", [BL, S, D], F32, kind="ExternalOutput").ap()

    with tile.TileContext(nc) as tc, ExitStack() as CTX, \
            nc.allow_low_precision(reason="bf16 matmul pipeline"):
        cst = CTX.enter_context(tc.tile_pool(name="cst", bufs=1))
        im = cst.tile([128, 128], F32, tag="im")
        nc.sync.dma_start(out=im, in_=imat_d)
        onesD = cst.tile([128, 1], BF16, tag="onesD")      # 1/D for LN stats
        nc.vector.memset(onesD, 1.0 / D)
        ones_col = cst.tile([128, 1], BF16, tag="onesc")   # denominators
        nc.vector.memset(ones_col, 1.0)
        onesPB = cst.tile([128, 128], BF16, tag="onespb")  # K=1 broadcasts
        nc.vector.memset(onesPB, 1.0)
        ones32 = cst.tile([128, 128], BF16, tag="ones32")  # ctxT fp8 scale
        nc.vector.memset(ones32, X8SCALE)
        epsb = cst.tile([1, 1], F32, tag="epsb")
        nc.vector.memset(epsb, EPS)
        lnsc = cst.tile([1, 1], F32, tag="lnsc")   # ln(X8SCALE) for fp8 xn
        nc.vector.memset(lnsc, float(np.log(X8SCALE)))
        zrob = cst.tile([1, 1], F32, tag="zrob")
        nc.vector.memset(zrob, 0.0)

        hp = CTX.enter_context(tc.tile_pool(name="hp", bufs=1))
        h = hp.tile([128, DT, NT], F32, tag="h")

        ap_pool = CTX.enter_context(tc.tile_pool(name="apool", bufs=1))   # qkv/g
        bp_pool = CTX.enter_context(tc.tile_pool(name="bpool", bufs=1))   # slot B
        op_pool = CTX.enter_context(tc.tile_pool(name="opool", bufs=1))   # out1
        wp = CTX.enter_context(tc.tile_pool(name="wp", bufs=2))           # weights
        psA = CTX.enter_context(tc.tile_pool(name="psA", bufs=4, space="PSUM"))
        psSc = CTX.enter_context(tc.tile_pool(name="psSc", bufs=4, space="PSUM"))

        # ---------------- input prep: h = (x + pe)^T feature-major ----------
        with ExitStack() as SP:
            pp = SP.enter_context(tc.tile_pool(name="prep", bufs=5))
            for b in range(BL):
                xt = []
                for st in range(4):
                    xpe = pp.tile([128, D], F32, tag="xpe", name=f"xpe{b}{st}")
                    nc.sync.dma_start(out=xpe, in_=x_d[b, st * 128:(st + 1) * 128, :])
                    pet = pp.tile([128, D], F32, tag="pet", name=f"pet{b}{st}")
                    nc.sync.dma_start(out=pet, in_=pe_d[st * 128:(st + 1) * 128, :])
                    nc.vector.tensor_add(xpe, xpe, pet)
                    xt.append(xpe)
                for d in range(DT):
                    ps = psA.tile([128, 512], F32, tag="pa", name=f"prtp{b}{d}")
                    for j in range(4):
                        nc.tensor.transpose(
                            ps[:, j * 128:(j + 1) * 128],
                            xt[j][:, d * 128:(d + 1) * 128], im)
                    nc.scalar.copy(h[:, d, b * 512:(b + 1) * 512], ps)

        # ---------------- helpers -------------------------------------------
        def layernorm(src, dst, li, g_d, b_d, use_aff, tagp, lnscale=None):
            """src [128,DT,NT] f32 -> dst [128,DT,NT] (normalized, cast to
            dst dtype). lnscale: ln(s) folded into the rstd exponent so the
            output is s * normalized (fp8 activation scaling); under use_aff
            the scale is instead folded host-side into g/b.
            """
            with ExitStack() as SL:
                spx = SL.enter_context(tc.tile_pool(name=f"lnx{tagp}", bufs=2))
                spc = SL.enter_context(tc.tile_pool(name=f"lnc{tagp}", bufs=1))
                if use_aff:
                    gsb = spc.tile([128, DT], F32, tag="gsb")
                    nc.sync.dma_start(out=gsb, in_=bass.AP(
                        tensor=g_d.tensor, offset=li * D, ap=[[1, 128], [128, DT]]))
                    bsb = spc.tile([128, DT], F32, tag="bsb")
                    nc.sync.dma_start(out=bsb, in_=bass.AP(
                        tensor=b_d.tensor, offset=li * D, ap=[[1, 128], [128, DT]]))
                for ch in range(2):
                    c = slice(ch * 512, (ch + 1) * 512)
                    xbf = spx.tile([128, DT, 512], BF16, tag="xbf",
                                   name=f"xbf{tagp}{ch}")
                    nc.vector.tensor_copy(out=xbf, in_=src[:, :, c])
                    xsq = spc.tile([128, DT, 512], BF16, tag="xsq",
                                   name=f"xsq{tagp}{ch}")
                    nc.vector.tensor_mul(xsq, xbf, xbf)
                    mup = psA.tile([1, 512], F32, tag="pa", name=f"sm{tagp}{ch}")
                    e2p = psA.tile([1, 512], F32, tag="pa", name=f"se{tagp}{ch}")
                    for k in range(DT):
                        nc.tensor.matmul(mup, onesD, xbf[:, k, :],
                                         start=(k == 0), stop=(k == DT - 1))
                    for k in range(DT):
                        nc.tensor.matmul(e2p, onesD, xsq[:, k, :],
                                         start=(k == 0), stop=(k == DT - 1))
                    mu = spc.tile([1, 512], F32, tag="mu", name=f"mu{tagp}{ch}")
                    nc.scalar.copy(mu, mup)
                    msq = spc.tile([1, 512], F32, tag="msq", name=f"mq{tagp}{ch}")
                    nc.vector.tensor_mul(msq, mu, mu)
                    var = spc.tile([1, 512], F32, tag="var", name=f"va{tagp}{ch}")
                    nc.vector.tensor_sub(var, e2p, msq)
                    lnv = spc.tile([1, 512], F32, tag="lnv", name=f"lv{tagp}{ch}")
                    nc.scalar.activation(lnv, var, AF.Ln, bias=epsb)
                    rstd = spc.tile([1, 512], BF16, tag="rstd",
                                    name=f"rs{tagp}{ch}")
                    rbias = zrob if (use_aff or lnscale is None) else lnscale
                    nc.scalar.activation(rstd, lnv, AF.Exp, scale=-0.5,
                                         bias=rbias)
                    ms = spc.tile([1, 512], BF16, tag="ms", name=f"msx{tagp}{ch}")
                    nc.vector.tensor_mul(ms, mu, rstd)
                    # broadcast rstd/ms to 128 partitions via K=1 matmuls
                    rb = psA.tile([128, 512], F32, tag="pa", name=f"rb{tagp}{ch}")
                    nc.tensor.matmul(rb, onesPB[0:1, :], rstd, start=True, stop=True)
                    mb = psA.tile([128, 512], F32, tag="pa", name=f"mb{tagp}{ch}")
                    nc.tensor.matmul(mb, onesPB[0:1, :], ms, start=True, stop=True)
                    rstd_b = spc.tile([128, 512], BF16, tag="rstdb",
                                      name=f"rsb{tagp}{ch}")
                    nc.scalar.copy(rstd_b, rb)
                    ms_b = spc.tile([128, 512], BF16, tag="msb",
                                    name=f"msb{tagp}{ch}")
                    nc.scalar.copy(ms_b, mb)
                    for k in range(DT):
                        t0 = spx.tile([128, 512], BF16, tag="t0",
                                      name=f"t0{tagp}{ch}{k}")
                        nc.vector.tensor_mul(t0, xbf[:, k, :], rstd_b)
                        if use_aff:
                            t1 = spx.tile([128, 512], BF16, tag="t1",
                                          name=f"t1{tagp}{ch}{k}")
                            nc.vector.tensor_sub(t1, t0, ms_b)
                            nc.scalar.activation(dst[:, k, c], t1, AF.Identity,
                                                 bias=bsb[:, k:k + 1],
                                                 scale=gsb[:, k:k + 1])
                        else:
                            nc.vector.tensor_sub(dst[:, k, c], t0, ms_b)

        def load_bias_row(d_ap, li, width, name):
            """bias row [width] -> [128, width//128] sbuf f32 (feature-major)."""
            t = wp.tile([128, width // 128], F32, tag="w", name=name)
            nc.sync.dma_start(out=t, in_=bass.AP(
                tensor=d_ap.tensor, offset=li * width,
                ap=[[1, 128], [128, width // 128]]))
            return t

        # ---------------- layers --------------------------------------------
        for i in range(L):
            # ---- LN1 -> xn (slot B), fp8 scaled by X8SCALE
            xn = bp_pool.tile([128, DT, NT], FP8, tag="B", name=f"xn{i}")
            layernorm(h, xn, i, ln1g_d if use_ln1 else None,
                      ln1b_d if use_ln1 else None, use_ln1, f"a{i}",
                      lnscale=lnsc)

            # ---- QKV projections -> qkv (slot A): q[0:8], k[8:16], v[16:24]
            qkv = ap_pool.tile([128, 24, NT], BF16, tag="A", name=f"qkv{i}")
            bq_sb = bk_sb = bv_sb = None
            if use_pbias:
                bq_sb = load_bias_row(pb_d["bq"], i, D, f"bq{i}")
                bk_sb = load_bias_row(pb_d["bk"], i, D, f"bk{i}")
                bv_sb = load_bias_row(pb_d["bv"], i, D, f"bv{i}")
            for wi, wname in enumerate(("wq", "wk")):
                bsb = (bq_sb, bk_sb)[wi]
                for hf in range(2):  # column half of the weight (single load)
                    wt = wp.tile([128, DT, 512], FP8, tag="w",
                                 name=f"{wname}{i}{hf}")
                    nc.sync.dma_start(out=wt, in_=w_d[wname][i, hf])
                    for ch in range(2):
                        cs = slice(ch * 512, (ch + 1) * 512)
                        for m in range(4):
                            mg = hf * 4 + m
                            pps = psA.tile([128, 512], F32, tag="pa",
                                           name=f"p{wname}{mg}{ch}")
                            for k2 in range(DT // 2):
                                nc.tensor.matmul(
                                    pps,
                                    wt[:, 2 * k2:2 * k2 + 2,
                                       m * 128:(m + 1) * 128],
                                    xn[:, 2 * k2:2 * k2 + 2, cs],
                                    start=(k2 == 0), stop=(k2 == DT // 2 - 1),
                                    perf_mode=DR)
                            dsl = qkv[:, wi * 8 + mg, cs]
                            if use_pbias:
                                nc.scalar.activation(dsl, pps, AF.Identity,
                                                     bias=bsb[:, mg:mg + 1],
                                                     scale=EVSCALE)
                            else:
                                nc.any.tensor_scalar_mul(dsl, pps, EVSCALE)
            # V: token-major out
            for hf in range(2):  # half of out-features
                wt = wp.tile([128, DT, 512], FP8, tag="w", name=f"wv{i}{hf}")
                nc.sync.dma_start(out=wt, in_=w_d["wv"][i, hf])
                for mt in range(DT):  # token tile
                    pps = psA.tile([128, 512], F32, tag="pa", name=f"pv{mt}{hf}")
                    for k2 in range(DT // 2):
                        nc.tensor.matmul(
                            pps,
                            xn[:, 2 * k2:2 * k2 + 2, mt * 128:(mt + 1) * 128],
                            wt[:, 2 * k2:2 * k2 + 2, :],
                            start=(k2 == 0), stop=(k2 == DT // 2 - 1),
                            perf_mode=DR)
                    dsl = qkv[:, 16 + mt, hf * 512:(hf + 1) * 512]
                    if use_pbias:
                        bvb = wp.tile([128, 512], F32, tag="w", name=f"bvb{i}{hf}")
                        nc.sync.dma_start(out=bvb, in_=bass.AP(
                            tensor=pb_d["bv"].tensor, offset=i * D + hf * 512,
                            ap=[[0, 128], [1, 512]]))
                        t8 = psA.tile([128, 512], F32, tag="pa",
                                      name=f"v8{mt}{hf}")
                        nc.vector.tensor_scalar_mul(t8, pps, EVSCALE)
                        nc.vector.tensor_add(dsl, t8, bvb)
                    else:
                        nc.any.tensor_scalar_mul(dsl, pps, EVSCALE)

            # ---- attention -> ctxT (slot B). b-outer so that WO of batch 0
            # can overlap attention of batch 1. Software-pipelined: scores +
            # bias + exp of iteration idx+1 are emitted before the context
            # matmuls of iteration idx, so the PE never drains waiting on ACT.
            ctxT = bp_pool.tile([128, DT, NT], FP8, tag="B", name=f"ctxT{i}")
            with ExitStack() as SA:
                gp_ = SA.enter_context(tc.tile_pool(name=f"gt{i}", bufs=2))
                ep_ = SA.enter_context(tc.tile_pool(name=f"exp{i}", bufs=3))
                rp_ = SA.enter_context(tc.tile_pool(name=f"rnv{i}", bufs=2))

                iters = [(b, th) for b in range(BL) for th in range(H // 2)]

                def stage1(b, th):
                    """scores + bias-add + exp for (b, th); returns exe pair."""
                    gtt = gp_.tile([128, 2, GW], BF16, tag="gt",
                                   name=f"gt{i}{b}{th}")
                    for hh2 in range(2):
                        nc.sync.dma_start(out=gtt[:, hh2, :],
                                          in_=gt_d[i, 2 * th + hh2])
                    # score matmuls (fp32 PSUM, row-strip packed head pair),
                    # then DVE bias-add -> SBUF bf16 exp-input tiles, then one
                    # big [128,2048] exp per head.
                    exs = [ep_.tile([128, 4, 512], BF16, tag="ex",
                                    name=f"ex{th}{b}{hh}") for hh in range(2)]
                    for kt in range(4):
                        scps = []
                        for hh in range(2):
                            ro = hh * 64
                            scp = psSc.tile([128, 512], F32, tag="sc",
                                            name=f"sc{th}{b}{hh}{kt}")
                            nc.tensor.matmul(
                                scp,
                                qkv[ro:ro + 64, 8 + th,
                                    b * 512 + kt * 128: b * 512 + (kt + 1) * 128],
                                qkv[ro:ro + 64, th, b * 512:(b + 1) * 512],
                                start=True, stop=True)
                            scps.append(scp)
                        gsl = slice(384 - kt * 128, 896 - kt * 128)
                        for hh in range(2):
                            nc.vector.tensor_add(
                                exs[hh][:, kt, :], scps[hh], gtt[:, hh, gsl])
                    exes = []
                    for hh in range(2):
                        exe = ep_.tile([128, 4, 512], BF16, tag="exe",
                                       name=f"exe{th}{b}{hh}")
                        nc.scalar.activation(exe, exs[hh], AF.Exp)
                        exes.append(exe)
                    return exes

                def stage2(b, th, exes):
                    """ctx + packed denominators + normalize for (b, th)."""
                    cps = psA.tile([128, 512], F32, tag="pa", name=f"c{th}{b}")
                    dns = psA.tile([128, 512], F32, tag="pa", name=f"d{th}{b}")
                    for kt in range(4):
                        for hh in range(2):
                            hi = 2 * th + hh
                            nc.tensor.matmul(
                                cps[hh * 64:hh * 64 + 64, :],
                                qkv[:, 16 + b * 4 + kt, hi * 64:(hi + 1) * 64],
                                exes[hh][:, kt, :],
                                start=(kt == 0), stop=(kt == 3))
                            nc.tensor.matmul(
                                dns[hh * 64:hh * 64 + 1, :], ones_col,
                                exes[hh][:, kt, :],
                                start=(kt == 0), stop=(kt == 3))
                    rv = rp_.tile([128, 512], BF16, tag="rv", name=f"rv{th}{b}")
                    for hh in range(2):
                        nc.vector.reciprocal(rv[hh * 64:hh * 64 + 1, :],
                                             dns[hh * 64:hh * 64 + 1, :])
                    rbp = psA.tile([128, 512], F32, tag="pa", name=f"rb{th}{b}")
                    for hh in range(2):
                        nc.tensor.matmul(
                            rbp[hh * 64:(hh + 1) * 64, :],
                            ones32[hh * 64:hh * 64 + 1, 0:64],
                            rv[hh * 64:hh * 64 + 1, :], start=True, stop=True)
                    rbs = rp_.tile([128, 512], BF16, tag="rbs", name=f"rs{th}{b}")
                    nc.scalar.copy(rbs, rbp)
                    nc.vector.tensor_mul(
                        ctxT[:, th, b * 512:(b + 1) * 512], cps, rbs)

                pending = None
                for (b, th) in iters:
                    exes = stage1(b, th)
                    if pending is not None:
                        stage2(*pending)
                    pending = (b, th, exes)
                stage2(*pending)

            # ---- out-projection + residual -> out1 (own slot, so WO of
            # ch0 can overlap attention of b=1 while qkv stays live)
            out1 = op_pool.tile([128, DT, NT], F32, tag="O", name=f"out1{i}")
            bo_sb = load_bias_row(ob_d["bo"], i, D, f"bo{i}") if use_obias else None
            for hf in range(2):
                wt = wp.tile([128, DT, 512], FP8, tag="w", name=f"wo{i}{hf}")
                nc.sync.dma_start(out=wt, in_=w_d["wo"][i, hf])
                for ch in range(2):  # ch0 first: overlaps attention of b=1
                    cs = slice(ch * 512, (ch + 1) * 512)
                    for m in range(4):
                        mg = hf * 4 + m
                        pps = psA.tile([128, 512], F32, tag="pa",
                                       name=f"po{mg}{ch}")
                        for k2 in range(DT // 2):
                            nc.tensor.matmul(
                                pps,
                                wt[:, 2 * k2:2 * k2 + 2, m * 128:(m + 1) * 128],
                                ctxT[:, 2 * k2:2 * k2 + 2, cs],
                                start=(k2 == 0), stop=(k2 == DT // 2 - 1),
                                perf_mode=DR)
                        dsl = out1[:, mg, cs]
                        if use_obias:
                            t = psA.tile([128, 512], F32, tag="pa",
                                         name=f"ob{mg}{ch}")
                            nc.scalar.activation(t, pps, AF.Identity,
                                                 bias=bo_sb[:, mg:mg + 1],
                                                 scale=EVSCALE)
                            nc.vector.tensor_add(dsl, t, h[:, mg, cs])
                        else:
                            # out1 = psum*EVSCALE + h in one fused DVE pass
                            nc.vector.scalar_tensor_tensor(
                                dsl, pps, EVSCALE, h[:, mg, cs],
                                op0=mybir.AluOpType.mult,
                                op1=mybir.AluOpType.add)

            # ---- LN2 -> xn2 (slot B)
            xn2 = bp_pool.tile([128, DT, NT], BF16, tag="B", name=f"xn2{i}")
            layernorm(out1, xn2, i, ln2g_d if use_ln2 else None,
                      ln2b_d if use_ln2 else None, use_ln2, f"b{i}")

            # ---- h += out1  (h becomes h_old + out1 = 2h + attn)
            for d in range(DT):
                nc.vector.tensor_add(h[:, d, :], h[:, d, :], out1[:, d, :])

            # ---- FFN: h += ffn(xn2); ch-outer so FFN2 ch0 overlaps FFN1 ch1
            b1_sb = load_bias_row(pb_d["b1"], i, F, f"b1{i}") if use_pbias else None
            b2_sb = load_bias_row(ob_d["b2"], i, D, f"b2{i}") if use_obias else None
            g = ap_pool.tile([128, FT, NT], BF16, tag="A", name=f"g{i}")
            for fb in range(8):
                wt = wp.tile([128, DT, 512], BF16, tag="w", name=f"w1{i}{fb}")
                nc.sync.dma_start(out=wt, in_=w1_d[i, fb])
                for ch in range(2):
                    cs = slice(ch * 512, (ch + 1) * 512)
                    for m in range(4):
                        fm = 4 * fb + m
                        pps = psA.tile([128, 512], F32, tag="pa",
                                       name=f"pf{fm}{ch}")
                        for k in range(DT):
                            nc.tensor.matmul(pps, wt[:, k, m * 128:(m + 1) * 128],
                                             xn2[:, k, cs],
                                             start=(k == 0), stop=(k == DT - 1))
                        gsl = g[:, fm, cs]
                        if use_pbias:
                            nc.scalar.activation(gsl, pps, AF.Gelu,
                                                 bias=b1_sb[:, fm:fm + 1])
                        else:
                            nc.scalar.activation(gsl, pps, AF.Gelu)
            for dm in range(DT):
                wt = wp.tile([128, FT, 128], BF16, tag="w", name=f"w2{i}{dm}")
                nc.sync.dma_start(out=wt, in_=w2_d[i, dm])
                for ch in range(2):
                    cs = slice(ch * 512, (ch + 1) * 512)
                    pps = psA.tile([128, 512], F32, tag="pa", name=f"pg{ch}{dm}")
                    for kf in range(FT):
                        nc.tensor.matmul(pps, wt[:, kf, :],
                                         g[:, kf, cs],
                                         start=(kf == 0), stop=(kf == FT - 1))
                    hsl = h[:, dm, cs]
                    if use_obias:
                        t = psA.tile([128, 512], F32, tag="pa", name=f"o2{ch}{dm}")
                        nc.scalar.activation(t, pps, AF.Identity,
                                             bias=b2_sb[:, dm:dm + 1])
                        nc.vector.tensor_add(hsl, hsl, t)
                    else:
                        nc.vector.tensor_add(hsl, hsl, pps)

        # ---------------- output: transpose h back to token-major -----------
        with ExitStack() as SO:
            op = SO.enter_context(tc.tile_pool(name="outp", bufs=2))
            for b in range(BL):
                for st in range(4):
                    ot = op.tile([128, D], F32, tag="ot", name=f"ot{b}{st}")
                    for half in range(2):
                        ps = psA.tile([128, 512], F32, tag="pa",
                                      name=f"otp{b}{st}{half}")
                        for j in range(4):
                            d = half * 4 + j
                            nc.tensor.transpose(
                                ps[:, j * 128:(j + 1) * 128],
                                h[:, d, b * 512 + st * 128: b * 512 + (st + 1) * 128],
                                im)
                        nc.scalar.copy(ot[:, half * 512:(half + 1) * 512], ps)
                    nc.sync.dma_start(
                        out=out_d[b, st * 128:(st + 1) * 128, :], in_=ot)

    nc.compile()
    return nc


def prepare(inputs):
    """Host-side prep: flags + per-core input maps."""
    x = np.asarray(inputs["x"], dtype=np.float32)
    pe = np.asarray(inputs["pe"], dtype=np.float32).reshape(P, D)[:S]
    bias_table = np.asarray(inputs["bias_table"], dtype=np.float32)

    use_pbias = any(np.any(np.asarray(inputs[k])) for k in ("bq", "bk", "bv", "b1"))
    use_obias = any(np.any(np.asarray(inputs[k])) for k in ("bo", "b2"))
    use_ln1 = (not np.all(np.asarray(inputs["ln1_g"]) == 1.0)) or \
        np.any(np.asarray(inputs["ln1_b"]))
    use_ln2 = (not np.all(np.asarray(inputs["ln2_g"]) == 1.0)) or \
        np.any(np.asarray(inputs["ln2_b"]))
    flags = (use_pbias, use_obias, use_ln1, use_ln2)

    # host-side weight relayout + bf16 cast (pure layout/dtype prep)
    wq = np.asarray(inputs["wq"], dtype=np.float32) * np.float32(QSCALE)
    wk = np.asarray(inputs["wk"], dtype=np.float32)
    wv = np.asarray(inputs["wv"], dtype=np.float32)
    wo = np.asarray(inputs["wo"], dtype=np.float32)
    w1 = np.asarray(inputs["w1"], dtype=np.float32)
    w2 = np.asarray(inputs["w2"], dtype=np.float32)

    def qk_layout(w):  # [L,D,D] -> [L,2,128,DT,512] per-partition contiguous
        # in-feature f = t*128 + p; out-feature = hf*512 + m; fp8 scaled
        return np.ascontiguousarray(
            (w * np.float32(W8SCALE))
            .reshape(L, DT, 128, 2, 512).transpose(0, 3, 2, 1, 4)).astype(F8)

    base = {
        "pe": np.ascontiguousarray(pe),
        "wq": qk_layout(wq), "wk": qk_layout(wk),
        "wv": qk_layout(wv), "wo": qk_layout(wo),
        # w1 [L,D,F]: f-out = fb*512 + (m*128+c) -> [L,8,128,DT,512]
        "w1": np.ascontiguousarray(
            w1.reshape(L, DT, 128, 8, 512).transpose(0, 3, 2, 1, 4)).astype(BF),
        # w2 [L,F,D]: in = kf*128+p, out = dm*128+c -> [L,DT,128,FT,128]
        "w2": np.ascontiguousarray(
            w2.reshape(L, FT, 128, DT, 128).transpose(0, 3, 2, 1, 4)).astype(BF),
        "imat": np.eye(128, dtype=np.float32),
    }
    # flipped Toeplitz windows: gt[l,h,p,j] = table[l, 127-p+j, h]
    tT = bias_table.transpose(0, 2, 1)          # [L,H,TBL]
    gt = np.empty((L, H, 128, GW), dtype=BF)
    for p in range(128):
        gt[:, :, p, :] = tT[:, :, 127 - p:127 - p + GW].astype(BF)
    base["gt"] = gt

    if use_pbias:
        base["b_bq"] = np.asarray(inputs["bq"], np.float32) * np.float32(QSCALE)
        base["b_bk"] = np.asarray(inputs["bk"], np.float32)
        base["b_bv"] = np.asarray(inputs["bv"], np.float32)
        base["b_b1"] = np.asarray(inputs["b1"], np.float32)
    if use_obias:
        base["b_bo"] = np.asarray(inputs["bo"], np.float32)
        base["b_b2"] = np.asarray(inputs["b2"], np.float32)
    if use_ln1:
        # xn is produced in fp8 scaled by X8SCALE; fold into the affine
        base["ln1_g"] = np.asarray(inputs["ln1_g"], np.float32) * np.float32(X8SCALE)
        base["ln1_b"] = np.asarray(inputs["ln1_b"], np.float32) * np.float32(X8SCALE)
    if use_ln2:
        base["ln2_g"] = np.asarray(inputs["ln2_g"], np.float32)
        base["ln2_b"] = np.asarray(inputs["ln2_b"], np.float32)

    in_maps = []
    for c in range(NCORES):
        m = dict(base)
        m["x"] = np.ascontiguousarray(x[c * BL:(c + 1) * BL])
        in_maps.append(m)
    return flags, in_maps


def get_nc(flags):
    if flags not in _CACHE:
        _CACHE[flags] = _build(flags)
    return _CACHE[flags]


def kernel(**inputs):
    flags, in_maps = prepare(inputs)
    nc = get_nc(flags)
    res = run_bass_kernel_spmd(nc, in_maps, core_ids=list(range(NCORES)))
    out = np.concatenate([r["out"] for r in res.results], axis=0)
    return out.astype(np.float32)


# revision 32
# speedup vs baseline: 1.1539x; 1.1539x over previous
"""Trainium2 Bass kernel for nn_Encoder_37915971289796 (6-layer transformer encoder).

Strategy: pure data-parallel over batch (B=16 -> 2 per core, 8 cores, no
collectives). Per core, activations live feature-major in SBUF; weights are
host-cast to bf16, host-relaid so every SBUF partition's data is one
contiguous 8KB HBM read, and streamed double-buffered; matmuls run bf16 with
fp32 PSUM accumulation.

v2 restructure vs the first working version:
- Attention: score matmuls write bf16 PSUM (half-width banks), the relative
  position bias is added by DVE at 2x bf16 rate straight into SBUF exp-input
  tiles; exp runs as two [128,2048] ACT calls per head pair; softmax
  denominators ride the context matmul stream as packed M=1 column-strip
  chains (no separate ones-matmul chains); a single reciprocal + one packed
  broadcast matmul pair + one multiply normalizes both heads. Score/exp work
  for iteration i+1 is emitted before the context matmuls of iteration i so
  the PE never waits on the scalar engine.
- LayerNorm: stats inputs are produced by DVE (bf16 copy + bf16 square at
  2x/4x modes) instead of big serial ACT passes; mean/E[x2] matmul chains are
  packed into one PSUM bank via column strips; 1/D is folded into the ones
  vector; normalization runs in bf16 at 2x DVE rate.
- Loop order: attention iterates b-outer so the output projection of the
  first 512 tokens overlaps attention of the second batch; QKV/FFN iterate
  ch-outer so each phase starts as soon as the first 512-token chunk of the
  previous phase is ready.
- QSCALE folded into wq host-side; ACT functions reduced to {Exp, Ln, Gelu,
  Copy} to minimize activation-table reloads.

Self-contained: hardcodes all shapes; takes FULL inputs, returns FULL output.
"""

import numpy as np
import ml_dtypes
from contextlib import ExitStack

import concourse.bass as bass
import concourse.mybir as mybir
import concourse.tile as tile
from concourse import bacc
from concourse.bass_utils import run_bass_kernel_spmd

F32 = mybir.dt.float32
BF16 = mybir.dt.bfloat16
FP8 = mybir.dt.float8e4
AF = mybir.ActivationFunctionType
DR = mybir.MatmulPerfMode.DoubleRow
BF = ml_dtypes.bfloat16
F8 = ml_dtypes.float8_e4m3fn if hasattr(ml_dtypes, "float8_e4m3fn") \
    else ml_dtypes.float8_e4m3
W8SCALE = 1024.0     # fp8 weight scale (power of 2, clip-safe)
X8SCALE = 32.0       # fp8 activation scale for xn / ctxT
EVSCALE = 1.0 / (W8SCALE * X8SCALE)   # folded into evacuations

L, D, H, F, S, B, P = 6, 1024, 16, 4096, 512, 16, 512
DH = D // H              # 64
NCORES = 8
BL = B // NCORES         # 2 batches per core
NT = BL * S              # 1024 tokens per core
DT = D // 128            # 8 d-tiles
FT = F // 128            # 32 f-tiles
TBL = 2 * P - 1          # 1023
GW = 896                 # flipped bias window width
EPS = 1e-6
QSCALE = 1.0 / float(np.sqrt(DH))

_CACHE = {}


def _build(flags):
    """Build the per-core Bass program. flags: (use_pbias, use_obias, use_ln1, use_ln2)"""
    use_pbias, use_obias, use_ln1, use_ln2 = flags
    nc = bacc.Bacc("TRN2", target_bir_lowering=False, debug=False)

    x_d = nc.dram_tensor("x", [BL, S, D], F32, kind="ExternalInput").ap()
    pe_d = nc.dram_tensor("pe", [S, D], F32, kind="ExternalInput").ap()
    # host-relayouted weights (bf16), per-partition contiguous:
    w_d = {}
    for w in ("wq", "wk", "wv", "wo"):
        w_d[w] = nc.dram_tensor(w, [L, 2, 128, DT, 512], FP8,
                                kind="ExternalInput").ap()
    w1_d = nc.dram_tensor("w1", [L, 8, 128, DT, 512], BF16,
                          kind="ExternalInput").ap()
    w2_d = nc.dram_tensor("w2", [L, DT, 128, FT, 128], BF16,
                          kind="ExternalInput").ap()
    # compact bias table rows, transposed: gt[l,h,:] = table[l,:,h]. The
    # partition-flipped Toeplitz windows are formed on-chip by a
    # negative-partition-stride DMA: partition p reads gt[l,h,127-p : 127-p+GW].
    gt_d = nc.dram_tensor("gt", [L, H, TBL], BF16, kind="ExternalInput").ap()
    imat_d = nc.dram_tensor("imat", [128, 128], F32, kind="ExternalInput").ap()
    if use_pbias:  # bq*QSCALE, bk, bv, b1 applied via ACT bias APs
        pb_d = {w: nc.dram_tensor(f"b_{w}", [L, D if w != "b1" else F], F32,
                                  kind="ExternalInput").ap()
                for w in ("bq", "bk", "bv", "b1")}
    if use_obias:  # bo, b2 applied via extra DVE passes
        ob_d = {w: nc.dram_tensor(f"b_{w}", [L, D], F32, kind="ExternalInput").ap()
                for w in ("bo", "b2")}
    if use_ln1:
        ln1g_d = nc.dram_tensor("ln1_g", [L, D], F32, kind="ExternalInput").ap()
        ln1b_d = nc.dram_tensor("ln1_b", [L, D], F32, kind="ExternalInput").ap()
    if use_ln2:
        ln2g_d = nc.dram_tensor("ln2_g", [L, D], F32, kind="ExternalInput").ap()
        ln2b_d = nc.dram_tensor("ln2_b", [L, D], F32, kind="ExternalInput").ap()
    out_d = nc.dram_tensor("out", [BL, S, D], F32, kind="ExternalOutput").ap()

    with tile.TileContext(nc) as tc, ExitStack() as CTX, \
            nc.allow_low_precision(reason="bf16 matmul pipeline"):
        cst = CTX.enter_context(tc.tile_pool(name="cst", bufs=1))
        im = cst.tile([128, 128], F32, tag="im")
        nc.sync.dma_start(out=im, in_=imat_d)
        onesD = cst.tile([128, 1], BF16, tag="onesD")      # 1/D for LN stats
        nc.vector.memset(onesD, 1.0 / D)
        ones_col = cst.tile([128, 1], BF16, tag="onesc")   # denominators
        nc.vector.memset(ones_col, 1.0)
        onesPB = cst.tile([128, 128], BF16, tag="onespb")  # K=1 broadcasts
        nc.vector.memset(onesPB, 1.0)
        ones32 = cst.tile([128, 128], BF16, tag="ones32")  # ctxT fp8 scale
        nc.vector.memset(ones32, X8SCALE)
        epsb = cst.tile([1, 1], F32, tag="epsb")
        nc.vector.memset(epsb, EPS)
        lnsc = cst.tile([1, 1], F32, tag="lnsc")   # ln(X8SCALE) for fp8 xn
        nc.vector.memset(lnsc, float(np.log(X8SCALE)))
        zrob = cst.tile([1, 1], F32, tag="zrob")
        nc.vector.memset(zrob, 0.0)

        hp = CTX.enter_context(tc.tile_pool(name="hp", bufs=1))
        h = hp.tile([128, DT, NT], F32, tag="h")

        ap_pool = CTX.enter_context(tc.tile_pool(name="apool", bufs=1))   # qkv/g
        bp_pool = CTX.enter_context(tc.tile_pool(name="bpool", bufs=1))   # slot B
        op_pool = CTX.enter_context(tc.tile_pool(name="opool", bufs=1))   # out1
        wp = CTX.enter_context(tc.tile_pool(name="wp", bufs=2))           # weights
        psA = CTX.enter_context(tc.tile_pool(name="psA", bufs=4, space="PSUM"))
        psSc = CTX.enter_context(tc.tile_pool(name="psSc", bufs=4, space="PSUM"))

        # ---------------- input prep: h = (x + pe)^T feature-major ----------
        with ExitStack() as SP:
            pp = SP.enter_context(tc.tile_pool(name="prep", bufs=5))
            for b in range(BL):
                xt = []
                for st in range(4):
                    xpe = pp.tile([128, D], F32, tag="xpe", name=f"xpe{b}{st}")
                    nc.sync.dma_start(out=xpe, in_=x_d[b, st * 128:(st + 1) * 128, :])
                    pet = pp.tile([128, D], F32, tag="pet", name=f"pet{b}{st}")
                    nc.sync.dma_start(out=pet, in_=pe_d[st * 128:(st + 1) * 128, :])
                    nc.vector.tensor_add(xpe, xpe, pet)
                    xt.append(xpe)
                for d in range(DT):
                    ps = psA.tile([128, 512], F32, tag="pa", name=f"prtp{b}{d}")
                    for j in range(4):
                        nc.tensor.transpose(
                            ps[:, j * 128:(j + 1) * 128],
                            xt[j][:, d * 128:(d + 1) * 128], im)
                    nc.scalar.copy(h[:, d, b * 512:(b + 1) * 512], ps)

        # ---------------- helpers -------------------------------------------
        def layernorm(src, dst, li, g_d, b_d, use_aff, tagp, lnscale=None):
            """src [128,DT,NT] f32 -> dst [128,DT,NT] (normalized, cast to
            dst dtype). lnscale: ln(s) folded into the rstd exponent so the
            output is s * normalized (fp8 activation scaling); under use_aff
            the scale is instead folded host-side into g/b.
            """
            with ExitStack() as SL:
                spx = SL.enter_context(tc.tile_pool(name=f"lnx{tagp}", bufs=2))
                spc = SL.enter_context(tc.tile_pool(name=f"lnc{tagp}", bufs=1))
                if use_aff:
                    gsb = spc.tile([128, DT], F32, tag="gsb")
                    nc.sync.dma_start(out=gsb, in_=bass.AP(
                        tensor=g_d.tensor, offset=li * D, ap=[[1, 128], [128, DT]]))
                    bsb = spc.tile([128, DT], F32, tag="bsb")
                    nc.sync.dma_start(out=bsb, in_=bass.AP(
                        tensor=b_d.tensor, offset=li * D, ap=[[1, 128], [128, DT]]))
                for ch in range(2):
                    c = slice(ch * 512, (ch + 1) * 512)
                    xbf = spx.tile([128, DT, 512], BF16, tag="xbf",
                                   name=f"xbf{tagp}{ch}")
                    nc.vector.tensor_copy(out=xbf, in_=src[:, :, c])
                    xsq = spc.tile([128, DT, 512], BF16, tag="xsq",
                                   name=f"xsq{tagp}{ch}")
                    nc.vector.tensor_mul(xsq, xbf, xbf)
                    mup = psA.tile([1, 512], F32, tag="pa", name=f"sm{tagp}{ch}")
                    e2p = psA.tile([1, 512], F32, tag="pa", name=f"se{tagp}{ch}")
                    for k in range(DT):
                        nc.tensor.matmul(mup, onesD, xbf[:, k, :],
                                         start=(k == 0), stop=(k == DT - 1))
                    for k in range(DT):
                        nc.tensor.matmul(e2p, onesD, xsq[:, k, :],
                                         start=(k == 0), stop=(k == DT - 1))
                    mu = spc.tile([1, 512], F32, tag="mu", name=f"mu{tagp}{ch}")
                    nc.scalar.copy(mu, mup)
                    msq = spc.tile([1, 512], F32, tag="msq", name=f"mq{tagp}{ch}")
                    nc.vector.tensor_mul(msq, mu, mu)
                    var = spc.tile([1, 512], F32, tag="var", name=f"va{tagp}{ch}")
                    nc.vector.tensor_sub(var, e2p, msq)
                    lnv = spc.tile([1, 512], F32, tag="lnv", name=f"lv{tagp}{ch}")
                    nc.scalar.activation(lnv, var, AF.Ln, bias=epsb)
                    rstd = spc.tile([1, 512], BF16, tag="rstd",
                                    name=f"rs{tagp}{ch}")
                    rbias = zrob if (use_aff or lnscale is None) else lnscale
                    nc.scalar.activation(rstd, lnv, AF.Exp, scale=-0.5,
                                         bias=rbias)
                    ms = spc.tile([1, 512], BF16, tag="ms", name=f"msx{tagp}{ch}")
                    nc.vector.tensor_mul(ms, mu, rstd)
                    # broadcast rstd/ms to 128 partitions via K=1 matmuls
                    rb = psA.tile([128, 512], F32, tag="pa", name=f"rb{tagp}{ch}")
                    nc.tensor.matmul(rb, onesPB[0:1, :], rstd, start=True, stop=True)
                    mb = psA.tile([128, 512], F32, tag="pa", name=f"mb{tagp}{ch}")
                    nc.tensor.matmul(mb, onesPB[0:1, :], ms, start=True, stop=True)
                    rstd_b = spc.tile([128, 512], BF16, tag="rstdb",
                                      name=f"rsb{tagp}{ch}")
                    nc.scalar.copy(rstd_b, rb)
                    ms_b = spc.tile([128, 512], BF16, tag="msb",
                                    name=f"msb{tagp}{ch}")
                    nc.scalar.copy(ms_b, mb)
                    for k in range(DT):
                        t0 = spx.tile([128, 512], BF16, tag="t0",
                                      name=f"t0{tagp}{ch}{k}")
                        nc.vector.tensor_mul(t0, xbf[:, k, :], rstd_b)
                        if use_aff:
                            t1 = spx.tile([128, 512], BF16, tag="t1",
                                          name=f"t1{tagp}{ch}{k}")
                            nc.vector.tensor_sub(t1, t0, ms_b)
                            nc.scalar.activation(dst[:, k, c], t1, AF.Identity,
                                                 bias=bsb[:, k:k + 1],
                                                 scale=gsb[:, k:k + 1])
                        else:
                            nc.vector.tensor_sub(dst[:, k, c], t0, ms_b)

        def load_bias_row(d_ap, li, width, name):
            """bias row [width] -> [128, width//128] sbuf f32 (feature-major)."""
            t = wp.tile([128, width // 128], F32, tag="w", name=name)
            nc.sync.dma_start(out=t, in_=bass.AP(
                tensor=d_ap.tensor, offset=li * width,
                ap=[[1, 128], [128, width // 128]]))
            return t

        # ---------------- layers --------------------------------------------
        for i in range(L):
            # ---- LN1 -> xn (slot B), fp8 scaled by X8SCALE
            xn = bp_pool.tile([128, DT, NT], FP8, tag="B", name=f"xn{i}")
            layernorm(h, xn, i, ln1g_d if use_ln1 else None,
                      ln1b_d if use_ln1 else None, use_ln1, f"a{i}",
                      lnscale=lnsc)

            # ---- QKV projections -> qkv (slot A): q[0:8], k[8:16], v[16:24]
            qkv = ap_pool.tile([128, 24, NT], BF16, tag="A", name=f"qkv{i}")
            bq_sb = bk_sb = bv_sb = None
            if use_pbias:
                bq_sb = load_bias_row(pb_d["bq"], i, D, f"bq{i}")
                bk_sb = load_bias_row(pb_d["bk"], i, D, f"bk{i}")
                bv_sb = load_bias_row(pb_d["bv"], i, D, f"bv{i}")
            for wi, wname in enumerate(("wq", "wk")):
                bsb = (bq_sb, bk_sb)[wi]
                for hf in range(2):  # column half of the weight (single load)
                    wt = wp.tile([128, DT, 512], FP8, tag="w",
                                 name=f"{wname}{i}{hf}")
                    nc.sync.dma_start(out=wt, in_=w_d[wname][i, hf])
                    for ch in range(2):
                        cs = slice(ch * 512, (ch + 1) * 512)
                        for m in range(4):
                            mg = hf * 4 + m
                            pps = psA.tile([128, 512], F32, tag="pa",
                                           name=f"p{wname}{mg}{ch}")
                            for k2 in range(DT // 2):
                                nc.tensor.matmul(
                                    pps,
                                    wt[:, 2 * k2:2 * k2 + 2,
                                       m * 128:(m + 1) * 128],
                                    xn[:, 2 * k2:2 * k2 + 2, cs],
                                    start=(k2 == 0), stop=(k2 == DT // 2 - 1),
                                    perf_mode=DR)
                            dsl = qkv[:, wi * 8 + mg, cs]
                            if use_pbias:
                                nc.scalar.activation(dsl, pps, AF.Identity,
                                                     bias=bsb[:, mg:mg + 1],
                                                     scale=EVSCALE)
                            else:
                                nc.any.tensor_scalar_mul(dsl, pps, EVSCALE)
            # V: token-major out
            for hf in range(2):  # half of out-features
                wt = wp.tile([128, DT, 512], FP8, tag="w", name=f"wv{i}{hf}")
                nc.sync.dma_start(out=wt, in_=w_d["wv"][i, hf])
                for mt in range(DT):  # token tile
                    pps = psA.tile([128, 512], F32, tag="pa", name=f"pv{mt}{hf}")
                    for k2 in range(DT // 2):
                        nc.tensor.matmul(
                            pps,
                            xn[:, 2 * k2:2 * k2 + 2, mt * 128:(mt + 1) * 128],
                            wt[:, 2 * k2:2 * k2 + 2, :],
                            start=(k2 == 0), stop=(k2 == DT // 2 - 1),
                            perf_mode=DR)
                    dsl = qkv[:, 16 + mt, hf * 512:(hf + 1) * 512]
                    if use_pbias:
                        bvb = wp.tile([128, 512], F32, tag="w", name=f"bvb{i}{hf}")
                        nc.sync.dma_start(out=bvb, in_=bass.AP(
                            tensor=pb_d["bv"].tensor, offset=i * D + hf * 512,
                            ap=[[0, 128], [1, 512]]))
                        t8 = psA.tile([128, 512], F32, tag="pa",
                                      name=f"v8{mt}{hf}")
                        nc.vector.tensor_scalar_mul(t8, pps, EVSCALE)
                        nc.vector.tensor_add(dsl, t8, bvb)
                    else:
                        nc.any.tensor_scalar_mul(dsl, pps, EVSCALE)

            # ---- attention -> ctxT (slot B). b-outer so that WO of batch 0
            # can overlap attention of batch 1. Software-pipelined: scores +
            # bias + exp of iteration idx+1 are emitted before the context
            # matmuls of iteration idx, so the PE never drains waiting on ACT.
            ctxT = bp_pool.tile([128, DT, NT], FP8, tag="B", name=f"ctxT{i}")
            with ExitStack() as SA:
                gp_ = SA.enter_context(tc.tile_pool(name=f"gt{i}", bufs=2))
                ep_ = SA.enter_context(tc.tile_pool(name=f"exp{i}", bufs=3))
                rp_ = SA.enter_context(tc.tile_pool(name=f"rnv{i}", bufs=2))

                iters = [(b, th) for b in range(BL) for th in range(H // 2)]

                def stage1(b, th):
                    """scores + bias-add + exp for (b, th); returns exe pair."""
                    gtt = gp_.tile([128, 2, GW], BF16, tag="gt",
                                   name=f"gt{i}{b}{th}")
                    for hh2 in range(2):
                        nc.sync.dma_start(out=gtt[:, hh2, :], in_=bass.AP(
                            tensor=gt_d.tensor,
                            offset=(i * H + 2 * th + hh2) * TBL + 127,
                            ap=[[-1, 128], [1, GW]]))
                    # score matmuls (fp32 PSUM, row-strip packed head pair),
                    # then DVE bias-add -> SBUF bf16 exp-input tiles, then one
                    # big [128,2048] exp per head.
                    exs = [ep_.tile([128, 4, 512], BF16, tag="ex",
                                    name=f"ex{th}{b}{hh}") for hh in range(2)]
                    for kt in range(4):
                        scps = []
                        for hh in range(2):
                            ro = hh * 64
                            scp = psSc.tile([128, 512], F32, tag="sc",
                                            name=f"sc{th}{b}{hh}{kt}")
                            nc.tensor.matmul(
                                scp,
                                qkv[ro:ro + 64, 8 + th,
                                    b * 512 + kt * 128: b * 512 + (kt + 1) * 128],
                                qkv[ro:ro + 64, th, b * 512:(b + 1) * 512],
                                start=True, stop=True)
                            scps.append(scp)
                        gsl = slice(384 - kt * 128, 896 - kt * 128)
                        for hh in range(2):
                            nc.vector.tensor_add(
                                exs[hh][:, kt, :], scps[hh], gtt[:, hh, gsl])
                    exes = []
                    for hh in range(2):
                        exe = ep_.tile([128, 4, 512], BF16, tag="exe",
                                       name=f"exe{th}{b}{hh}")
                        nc.scalar.activation(exe, exs[hh], AF.Exp)
                        exes.append(exe)
                    return exes

                def stage2(b, th, exes):
                    """ctx + packed denominators + normalize for (b, th)."""
                    cps = psA.tile([128, 512], F32, tag="pa", name=f"c{th}{b}")
                    dns = psA.tile([128, 512], F32, tag="pa", name=f"d{th}{b}")
                    for kt in range(4):
                        for hh in range(2):
                            hi = 2 * th + hh
                            nc.tensor.matmul(
                                cps[hh * 64:hh * 64 + 64, :],
                                qkv[:, 16 + b * 4 + kt, hi * 64:(hi + 1) * 64],
                                exes[hh][:, kt, :],
                                start=(kt == 0), stop=(kt == 3))
                            nc.tensor.matmul(
                                dns[hh * 64:hh * 64 + 1, :], ones_col,
                                exes[hh][:, kt, :],
                                start=(kt == 0), stop=(kt == 3))
                    rv = rp_.tile([128, 512], BF16, tag="rv", name=f"rv{th}{b}")
                    for hh in range(2):
                        nc.vector.reciprocal(rv[hh * 64:hh * 64 + 1, :],
                                             dns[hh * 64:hh * 64 + 1, :])
                    rbp = psA.tile([128, 512], F32, tag="pa", name=f"rb{th}{b}")
                    for hh in range(2):
                        nc.tensor.matmul(
                            rbp[hh * 64:(hh + 1) * 64, :],
                            ones32[hh * 64:hh * 64 + 1, 0:64],
                            rv[hh * 64:hh * 64 + 1, :], start=True, stop=True)
                    rbs = rp_.tile([128, 512], BF16, tag="rbs", name=f"rs{th}{b}")
                    nc.scalar.copy(rbs, rbp)
                    nc.vector.tensor_mul(
                        ctxT[:, th, b * 512:(b + 1) * 512], cps, rbs)

                pending = None
                for (b, th) in iters:
                    exes = stage1(b, th)
                    if pending is not None:
                        stage2(*pending)
                    pending = (b, th, exes)
                stage2(*pending)

            # ---- out-projection + residual -> out1 (own slot, so WO of
            # ch0 can overlap attention of b=1 while qkv stays live)
            out1 = op_pool.tile([128, DT, NT], F32, tag="O", name=f"out1{i}")
            bo_sb = load_bias_row(ob_d["bo"], i, D, f"bo{i}") if use_obias else None
            for hf in range(2):
                wt = wp.tile([128, DT, 512], FP8, tag="w", name=f"wo{i}{hf}")
                nc.sync.dma_start(out=wt, in_=w_d["wo"][i, hf])
                for ch in range(2):  # ch0 first: overlaps attention of b=1
                    cs = slice(ch * 512, (ch + 1) * 512)
                    for m in range(4):
                        mg = hf * 4 + m
                        pps = psA.tile([128, 512], F32, tag="pa",
                                       name=f"po{mg}{ch}")
                        for k2 in range(DT // 2):
                            nc.tensor.matmul(
                                pps,
                                wt[:, 2 * k2:2 * k2 + 2, m * 128:(m + 1) * 128],
                                ctxT[:, 2 * k2:2 * k2 + 2, cs],
                                start=(k2 == 0), stop=(k2 == DT // 2 - 1),
                                perf_mode=DR)
                        dsl = out1[:, mg, cs]
                        if use_obias:
                            t = psA.tile([128, 512], F32, tag="pa",
                                         name=f"ob{mg}{ch}")
                            nc.scalar.activation(t, pps, AF.Identity,
                                                 bias=bo_sb[:, mg:mg + 1],
                                                 scale=EVSCALE)
                            nc.vector.tensor_add(dsl, t, h[:, mg, cs])
                        else:
                            # out1 = psum*EVSCALE + h in one fused DVE pass
                            nc.vector.scalar_tensor_tensor(
                                dsl, pps, EVSCALE, h[:, mg, cs],
                                op0=mybir.AluOpType.mult,
                                op1=mybir.AluOpType.add)

            # ---- LN2 -> xn2 (slot B)
            xn2 = bp_pool.tile([128, DT, NT], BF16, tag="B", name=f"xn2{i}")
            layernorm(out1, xn2, i, ln2g_d if use_ln2 else None,
                      ln2b_d if use_ln2 else None, use_ln2, f"b{i}")

            # ---- h += out1  (h becomes h_old + out1 = 2h + attn)
            for d in range(DT):
                nc.vector.tensor_add(h[:, d, :], h[:, d, :], out1[:, d, :])

            # ---- FFN: h += ffn(xn2); ch-outer so FFN2 ch0 overlaps FFN1 ch1
            b1_sb = load_bias_row(pb_d["b1"], i, F, f"b1{i}") if use_pbias else None
            b2_sb = load_bias_row(ob_d["b2"], i, D, f"b2{i}") if use_obias else None
            g = ap_pool.tile([128, FT, NT], BF16, tag="A", name=f"g{i}")
            for fb in range(8):
                wt = wp.tile([128, DT, 512], BF16, tag="w", name=f"w1{i}{fb}")
                nc.sync.dma_start(out=wt, in_=w1_d[i, fb])
                for ch in range(2):
                    cs = slice(ch * 512, (ch + 1) * 512)
                    for m in range(4):
                        fm = 4 * fb + m
                        pps = psA.tile([128, 512], F32, tag="pa",
                                       name=f"pf{fm}{ch}")
                        for k in range(DT):
                            nc.tensor.matmul(pps, wt[:, k, m * 128:(m + 1) * 128],
                                             xn2[:, k, cs],
                                             start=(k == 0), stop=(k == DT - 1))
                        gsl = g[:, fm, cs]
                        if use_pbias:
                            nc.scalar.activation(gsl, pps, AF.Gelu,
                                                 bias=b1_sb[:, fm:fm + 1])
                        else:
                            nc.scalar.activation(gsl, pps, AF.Gelu)
            for dm in range(DT):
                wt = wp.tile([128, FT, 128], BF16, tag="w", name=f"w2{i}{dm}")
                nc.sync.dma_start(out=wt, in_=w2_d[i, dm])
                for ch in range(2):
                    cs = slice(ch * 512, (ch + 1) * 512)
                    pps = psA.tile([128, 512], F32, tag="pa", name=f"pg{ch}{dm}")
                    for kf in range(FT):
                        nc.tensor.matmul(pps, wt[:, kf, :],
                                         g[:, kf, cs],
                                         start=(kf == 0), stop=(kf == FT - 1))
                    hsl = h[:, dm, cs]
                    if use_obias:
                        t = psA.tile([128, 512], F32, tag="pa", name=f"o2{ch}{dm}")
                        nc.scalar.activation(t, pps, AF.Identity,
                                             bias=b2_sb[:, dm:dm + 1])
                        nc.vector.tensor_add(hsl, hsl, t)
                    else:
                        nc.vector.tensor_add(hsl, hsl, pps)

        # ---------------- output: transpose h back to token-major -----------
        with ExitStack() as SO:
            op = SO.enter_context(tc.tile_pool(name="outp", bufs=2))
            for b in range(BL):
                for st in range(4):
                    ot = op.tile([128, D], F32, tag="ot", name=f"ot{b}{st}")
                    for half in range(2):
                        ps = psA.tile([128, 512], F32, tag="pa",
                                      name=f"otp{b}{st}{half}")
                        for j in range(4):
                            d = half * 4 + j
                            nc.tensor.transpose(
                                ps[:, j * 128:(j + 1) * 128],
                                h[:, d, b * 512 + st * 128: b * 512 + (st + 1) * 128],
                                im)
                        nc.scalar.copy(ot[:, half * 512:(half + 1) * 512], ps)
                    nc.sync.dma_start(
                        out=out_d[b, st * 128:(st + 1) * 128, :], in_=ot)

    nc.compile()
    return nc


def prepare(inputs):
    """Host-side prep: flags + per-core input maps."""
    x = np.asarray(inputs["x"], dtype=np.float32)
    pe = np.asarray(inputs["pe"], dtype=np.float32).reshape(P, D)[:S]
    bias_table = np.asarray(inputs["bias_table"], dtype=np.float32)

    use_pbias = any(np.any(np.asarray(inputs[k])) for k in ("bq", "bk", "bv", "b1"))
    use_obias = any(np.any(np.asarray(inputs[k])) for k in ("bo", "b2"))
    use_ln1 = (not np.all(np.asarray(inputs["ln1_g"]) == 1.0)) or \
        np.any(np.asarray(inputs["ln1_b"]))
    use_ln2 = (not np.all(np.asarray(inputs["ln2_g"]) == 1.0)) or \
        np.any(np.asarray(inputs["ln2_b"]))
    flags = (use_pbias, use_obias, use_ln1, use_ln2)

    # host-side weight relayout + bf16 cast (pure layout/dtype prep)
    wq = np.asarray(inputs["wq"], dtype=np.float32) * np.float32(QSCALE)
    wk = np.asarray(inputs["wk"], dtype=np.float32)
    wv = np.asarray(inputs["wv"], dtype=np.float32)
    wo = np.asarray(inputs["wo"], dtype=np.float32)
    w1 = np.asarray(inputs["w1"], dtype=np.float32)
    w2 = np.asarray(inputs["w2"], dtype=np.float32)

    def qk_layout(w):  # [L,D,D] -> [L,2,128,DT,512] per-partition contiguous
        # in-feature f = t*128 + p; out-feature = hf*512 + m; fp8 scaled
        return np.ascontiguousarray(
            (w * np.float32(W8SCALE))
            .reshape(L, DT, 128, 2, 512).transpose(0, 3, 2, 1, 4)).astype(F8)

    base = {
        "pe": np.ascontiguousarray(pe),
        "wq": qk_layout(wq), "wk": qk_layout(wk),
        "wv": qk_layout(wv), "wo": qk_layout(wo),
        # w1 [L,D,F]: f-out = fb*512 + (m*128+c) -> [L,8,128,DT,512]
        "w1": np.ascontiguousarray(
            w1.reshape(L, DT, 128, 8, 512).transpose(0, 3, 2, 1, 4)).astype(BF),
        # w2 [L,F,D]: in = kf*128+p, out = dm*128+c -> [L,DT,128,FT,128]
        "w2": np.ascontiguousarray(
            w2.reshape(L, FT, 128, DT, 128).transpose(0, 3, 2, 1, 4)).astype(BF),
        "imat": np.eye(128, dtype=np.float32),
    }
    # compact transposed bias table; windows are formed on-chip
    base["gt"] = np.ascontiguousarray(
        bias_table.transpose(0, 2, 1)).astype(BF)   # [L,H,TBL]

    if use_pbias:
        base["b_bq"] = np.asarray(inputs["bq"], np.float32) * np.float32(QSCALE)
        base["b_bk"] = np.asarray(inputs["bk"], np.float32)
        base["b_bv"] = np.asarray(inputs["bv"], np.float32)
        base["b_b1"] = np.asarray(inputs["b1"], np.float32)
    if use_obias:
        base["b_bo"] = np.asarray(inputs["bo"], np.float32)
        base["b_b2"] = np.asarray(inputs["b2"], np.float32)
    if use_ln1:
        # xn is produced in fp8 scaled by X8SCALE; fold into the affine
        base["ln1_g"] = np.asarray(inputs["ln1_g"], np.float32) * np.float32(X8SCALE)
        base["ln1_b"] = np.asarray(inputs["ln1_b"], np.float32) * np.float32(X8SCALE)
    if use_ln2:
        base["ln2_g"] = np.asarray(inputs["ln2_g"], np.float32)
        base["ln2_b"] = np.asarray(inputs["ln2_b"], np.float32)

    in_maps = []
    for c in range(NCORES):
        m = dict(base)
        m["x"] = np.ascontiguousarray(x[c * BL:(c + 1) * BL])
        in_maps.append(m)
    return flags, in_maps


def get_nc(flags):
    if flags not in _CACHE:
        _CACHE[flags] = _build(flags)
    return _CACHE[flags]


def kernel(**inputs):
    flags, in_maps = prepare(inputs)
    nc = get_nc(flags)
    res = run_bass_kernel_spmd(nc, in_maps, core_ids=list(range(NCORES)))
    out = np.concatenate([r["out"] for r in res.results], axis=0)
    return out.astype(np.float32)
